# revision 1
# baseline (speedup 1.0000x reference)
"""Trainium2 Bass kernel for ContextHyperLinearSSM.

Computes out[b,:] = x[b,:] @ (WA[context[b]] * adj_xx) + u[b,:] @ (WB[context[b]] * adj_xu)

Strategy: shard the CONTEXT axis across the 8 cores (64 contexts each).
The host groups samples by context (padded to the max group size G), so each
core streams its 64 contexts' weight banks from HBM exactly once, applies the
adjacency masks on-device, and runs 3 accumulating matmuls per context.
Each sample's row is computed by exactly one core, so the host-side unshard
is a pure scatter.

Device-side layout: contexts are processed in groups of CT; each half-group's
payload (B-weights, A-weights, x/u activations) is packed by the host into a
single contiguous HBM blob so one DMA per half-group runs at full descriptor
efficiency.  A single in-place DVE multiply against a combined [adjB|adjA]
mask tile masks a half-group's weights.  All CT contexts of a group accumulate
into one PSUM bank (two 64-aligned partition slots x two free halves), so one
ACT copy per bank drains PSUM.
"""

import sys

sys.path.insert(0, "/opt/trn_rl_repo")

import numpy as np

import concourse.bass as bass
import concourse.mybir as mybir
import concourse.tile as tile
from concourse import bacc
from concourse.bass_utils import run_bass_kernel_spmd

N_CORES = 8
CT = 8  # contexts per PSUM group
W_BUFS = 4

# matmul operand dtype: float32 (exact) or float32r (tf32-like, 4x PE rate)
MM_DT = mybir.dt.float32


def _install_profile_shim():
    """Register the NTFF profile hook that trn_boot skips when
    antenv.axon_hooks is missing from the image (profiling only)."""
    import types
    if "antenv.axon_hooks" in sys.modules:
        return
    try:
        from trn_agent_boot.trn_boot import _ntff_profile_via_ctypes
        hook = _ntff_profile_via_ctypes("/opt/axon/libaxon_pjrt.so")
    except Exception:
        hook = None
    mod = types.ModuleType("antenv.axon_hooks")
    mod.get_axon_ntff_profile_hook = lambda: hook
    mod.set_axon_ntff_profile_hook = lambda h: None
    sys.modules["antenv.axon_hooks"] = mod


def _build_program(CP, S, A, G):
    """Build the per-core Bass program. CP contexts/core, group size G."""
    f32 = mybir.dt.float32
    nc = bacc.Bacc("TRN2", target_bir_lowering=False)

    HS = S // 128  # 128-row K-chunks of the A contraction
    K = HS + 1     # matmuls per context (1 B-term + HS A-terms)
    assert S % 128 == 0 and A == 128
    NG = CP // CT
    CH = CT // 2   # contexts per half-group payload
    assert CP % CT == 0 and CT % 2 == 0
    WF = CH * K * S   # weight f32 per partition line per half-group
    AF = CH * K * G   # activation f32 per partition line per half-group

    # PSUM packing: FF contexts along the free dim of a bank, two 64-aligned
    # partition slots (matmul out base partition must be 0/32/64)
    FF = max(1, min(CT, 512 // S))
    PSL = 2 if G <= 64 else 1
    CPT = min(CT, PSL * FF)
    T = -(-CT // CPT)
    assert T * CPT == CT, (CT, FF, PSL, CPT)

    blob = nc.dram_tensor("blob", [NG, 2, 128, WF + AF], f32,
                          kind="ExternalInput").ap()
    adj_xx = nc.dram_tensor("adj_xx", [HS, 128, S], mybir.dt.uint8,
                            kind="ExternalInput").ap()
    adj_xu = nc.dram_tensor("adj_xu", [A, S], mybir.dt.uint8,
                            kind="ExternalInput").ap()
    # output blob: [group][partition-slot][sample][bank][context-half][s]
    out = nc.dram_tensor("out", [NG, PSL, G, T, FF, S], f32,
                         kind="ExternalOutput").ap()

    rounded = MM_DT == mybir.dt.float32r

    with tile.TileContext(nc) as tc:
        with (
            tc.tile_pool(name="const", bufs=1) as const,
            tc.tile_pool(name="w", bufs=W_BUFS) as wpool,
            tc.tile_pool(name="o", bufs=3) as opool,
            tc.tile_pool(name="psum", bufs=8, space="PSUM") as psum,
        ):
            # combined [adjB | adjA] mask: raw u8 over the fast HWDGE ring,
            # then one DVE cast-copy (also the same-engine funnel for the
            # mask TTs).  SWDGE cast-DMAs would cost ~17us of ramp-in.
            adjU = const.tile([128, K * S], mybir.dt.uint8)
            nc.sync.dma_start(adjU[:, :S], adj_xu[:])
            nc.sync.dma_start(
                adjU[:, S:].rearrange("p (h s) -> p h s", h=HS),
                adj_xx.rearrange("h p s -> p h s"))
            adjC = const.tile([128, K * S], f32)
            nc.vector.tensor_copy(adjC[:], adjU[:])
            adjC_b = adjC[:, None, :].to_broadcast([128, CH, K * S])

            for g in range(NG):
                halves = []
                for hf in range(2):
                    hb = wpool.tile([128, WF + AF], f32, tag="hb",
                                    name=f"hb_{g}_{hf}")
                    nc.sync.dma_start(hb[:], blob[g, hf])
                    wv = hb[:, :WF].rearrange("p (c k s) -> p c k s",
                                              c=CH, k=K)
                    av = hb[:, WF:].rearrange("p (c k g) -> p c k g",
                                              c=CH, k=K)
                    if rounded:
                        wm = wpool.tile([128, WF], MM_DT, tag="wm",
                                        name=f"wm_{g}_{hf}")
                        am = wpool.tile([128, AF], MM_DT, tag="am",
                                        name=f"am_{g}_{hf}")
                        nc.vector.tensor_copy(am[:], hb[:, WF:])
                        nc.vector.tensor_tensor(
                            wm[:].rearrange("p (c ks) -> p c ks", c=CH),
                            hb[:, :WF].rearrange("p (c ks) -> p c ks", c=CH),
                            adjC_b, mybir.AluOpType.mult)
                        wv = wm[:].rearrange("p (c k s) -> p c k s",
                                             c=CH, k=K)
                        av = am[:].rearrange("p (c k g) -> p c k g",
                                             c=CH, k=K)
                    else:
                        # mask B+A weights with ONE in-place multiply
                        nc.vector.tensor_tensor(
                            hb[:, :WF].rearrange("p (c ks) -> p c ks", c=CH),
                            hb[:, :WF].rearrange("p (c ks) -> p c ks", c=CH),
                            adjC_b, mybir.AluOpType.mult)
                    halves.append((wv, av))

                ps_tiles = [psum.tile([128, FF * S], f32, tag="ps",
                                      name=f"ps_{g}_{t}")
                            for t in range(T)]
                for c in range(CT):
                    hf, ci = divmod(c, CH)
                    wv, av = halves[hf]
                    t, r2 = divmod(c, CPT)
                    sl, cf = divmod(r2, FF)
                    pslice = ps_tiles[t][sl * 64:sl * 64 + G,
                                         cf * S:cf * S + S]
                    for k in range(K):
                        nc.tensor.matmul(
                            pslice,
                            lhsT=av[:, ci, k, :],
                            rhs=wv[:, ci, k, :],
                            start=(k == 0), stop=(k == K - 1))
                out_sb = opool.tile([128, T, FF, S], f32)
                for t in range(T):
                    for sl in range(PSL):
                        nc.scalar.copy(
                            out_sb[sl * 64:sl * 64 + G, t].rearrange(
                                "p f s -> p (f s)"),
                            ps_tiles[t][sl * 64:sl * 64 + G, :])
                for sl in range(PSL):
                    nc.scalar.dma_start(
                        out[g, sl], out_sb[sl * 64:sl * 64 + G])

    nc.compile()
    return nc


def kernel(x, u, WA, WB, adj_xx, adj_xu, context, _trace=False):
    B, S = x.shape
    _, A = u.shape
    C = WA.shape[0]
    assert C % N_CORES == 0
    CP = C // N_CORES
    HS = S // 128
    K = HS + 1
    NG = CP // CT
    CH = CT // 2

    # ---- host-side shard: group samples by context --------------------
    context = np.asarray(context)
    cnt = np.bincount(context, minlength=C)
    G = int(cnt.max())
    G = max(4, ((G + 3) // 4) * 4)
    order = np.argsort(context, kind="stable")
    starts = np.zeros(C + 1, np.int64)
    starts[1:] = np.cumsum(cnt)
    j = np.arange(G)
    valid = j[None, :] < cnt[:, None]                      # [C, G]
    pos = starts[:-1, None] + np.minimum(j[None, :],
                                         np.maximum(cnt[:, None] - 1, 0))
    gidx = order[pos]                                      # [C, G]

    Xp = np.asarray(x, np.float32)[gidx]                   # [C, G, S]
    Up = np.asarray(u, np.float32)[gidx]                   # [C, G, A]
    XpT = np.ascontiguousarray(Xp.transpose(0, 2, 1))      # [C, S, G]
    UpT = np.ascontiguousarray(Up.transpose(0, 2, 1))      # [C, A, G]

    WA = np.ascontiguousarray(WA, np.float32)
    WB = np.ascontiguousarray(WB, np.float32)
    adjxx_u8 = np.ascontiguousarray(adj_xx).view(np.uint8).reshape(HS, 128, S)
    adjxu_u8 = np.ascontiguousarray(adj_xu).view(np.uint8)

    WF = CH * K * S
    AF = CH * K * G
    in_maps = []
    for k in range(N_CORES):
        sl = slice(k * CP, (k + 1) * CP)
        # pack each half-group's weights+activations into one contiguous
        # blob: per partition line [CH, K, S] weights then [CH, K, G] acts,
        # slot 0 = B-term, slot 1+h = A-term K-chunk h
        W3 = np.empty((NG, 2, 128, CH, K, S), np.float32)
        W3[..., 0, :] = WB[sl].reshape(NG, 2, CH, 128, S) \
            .transpose(0, 1, 3, 2, 4)
        W3[..., 1:, :] = WA[sl].reshape(NG, 2, CH, HS, 128, S) \
            .transpose(0, 1, 4, 2, 3, 5)
        A3 = np.empty((NG, 2, 128, CH, K, G), np.float32)
        A3[..., 0, :] = UpT[sl].reshape(NG, 2, CH, 128, G) \
            .transpose(0, 1, 3, 2, 4)
        A3[..., 1:, :] = XpT[sl].reshape(NG, 2, CH, HS, 128, G) \
            .transpose(0, 1, 4, 2, 3, 5)
        blob = np.concatenate(
            [W3.reshape(NG, 2, 128, WF), A3.reshape(NG, 2, 128, AF)],
            axis=-1)
        in_maps.append({
            "blob": np.ascontiguousarray(blob),
            "adj_xx": adjxx_u8,
            "adj_xu": adjxu_u8,
        })

    if _trace:
        _install_profile_shim()
    nc = _build_program(CP, S, A, G)
    res = run_bass_kernel_spmd(nc, in_maps, core_ids=list(range(N_CORES)),
                               trace=_trace)

    # device output blobs [NG, PSL, G, T, FF, S] -> [CP, G, S].
    # context c in a group lives at bank t=c//CPT, partition slot
    # sl=(c%CPT)//FF (64-aligned), free half cf=c%FF.
    outs = []
    for r in res.results:
        v = r["out"]
        # axes (g, sl, gg, t, cf, s) -> (g, t, sl, cf, gg, s)
        v = v.transpose(0, 3, 1, 4, 2, 5).reshape(CP, G, S)
        outs.append(v)
    Out_all = np.concatenate(outs, axis=0)                 # [C, G, S]
    out_full = np.zeros((B, S), np.float32)
    out_full[gidx[valid]] = Out_all[valid]

    if _trace:
        return out_full, res
    return out_full



# revision 6
# speedup vs baseline: 1.6310x; 1.6310x over previous
"""Trainium2 Bass kernel for ContextHyperLinearSSM.

Computes out[b,:] = x[b,:] @ (WA[context[b]] * adj_xx) + u[b,:] @ (WB[context[b]] * adj_xu)

Strategy: shard the CONTEXT axis across the 8 cores (64 contexts each).
The host groups samples by context (padded to the max group size G), so each
core streams its 64 contexts' weight banks from HBM exactly once, applies the
adjacency masks on-device, and runs 3 accumulating matmuls per context.
Each sample's row is computed by exactly one core, so the host-side unshard
is a pure scatter.

Device-side layout: contexts are processed in groups of CT; each half-group's
payload (B-weights, A-weights, x/u activations) is packed by the host into a
single contiguous HBM blob so one DMA per half-group runs at full descriptor
efficiency.  A single in-place DVE multiply against a combined [adjB|adjA]
mask tile masks a half-group's weights.  All CT contexts of a group accumulate
into one PSUM bank (two 64-aligned partition slots x two free halves), so one
ACT copy per bank drains PSUM.
"""

import sys

sys.path.insert(0, "/opt/trn_rl_repo")

import ml_dtypes
import numpy as np

import concourse.bass as bass
import concourse.mybir as mybir
import concourse.tile as tile
from concourse import bacc
from concourse.bass_utils import run_bass_kernel_spmd

N_CORES = 8
CT = 8  # contexts per PSUM group
W_BUFS = 4

# blob / matmul operand dtype: bf16 halves HBM traffic and streams the PE
# at 4x the f32 rate; PSUM still accumulates in f32 (rel err ~2e-3 << 2e-2)
IO_DT = mybir.dt.bfloat16


def _install_profile_shim():
    """Register the NTFF profile hook that trn_boot skips when
    antenv.axon_hooks is missing from the image (profiling only)."""
    import types
    if "antenv.axon_hooks" in sys.modules:
        return
    try:
        from trn_agent_boot.trn_boot import _ntff_profile_via_ctypes
        hook = _ntff_profile_via_ctypes("/opt/axon/libaxon_pjrt.so")
    except Exception:
        hook = None
    mod = types.ModuleType("antenv.axon_hooks")
    mod.get_axon_ntff_profile_hook = lambda: hook
    mod.set_axon_ntff_profile_hook = lambda h: None
    sys.modules["antenv.axon_hooks"] = mod


def _build_program(CP, S, A, G):
    """Build the per-core Bass program. CP contexts/core, group size G."""
    f32 = mybir.dt.float32
    nc = bacc.Bacc("TRN2", target_bir_lowering=False)

    HS = S // 128  # 128-row K-chunks of the A contraction
    K = HS + 1     # matmuls per context (1 B-term + HS A-terms)
    assert S % 128 == 0 and A == 128
    NG = CP // CT
    CH = CT // 2   # contexts per half-group payload
    assert CP % CT == 0 and CT % 2 == 0
    WF = CH * K * S   # weight f32 per partition line per half-group
    AF = CH * K * G   # activation f32 per partition line per half-group

    # PSUM packing: FF contexts along the free dim of a bank, two 64-aligned
    # partition slots (matmul out base partition must be 0/32/64)
    FF = max(1, min(CT, 512 // S))
    PSL = 2 if G <= 64 else 1
    CPT = min(CT, PSL * FF)
    T = -(-CT // CPT)
    assert T * CPT == CT, (CT, FF, PSL, CPT)

    blob = nc.dram_tensor("blob", [NG, 2, 128, WF + AF], IO_DT,
                          kind="ExternalInput").ap()
    adj_xx = nc.dram_tensor("adj_xx", [HS, 128, S], mybir.dt.uint8,
                            kind="ExternalInput").ap()
    adj_xu = nc.dram_tensor("adj_xu", [A, S], mybir.dt.uint8,
                            kind="ExternalInput").ap()
    # output blob: [group][partition-slot][sample][bank][context-half][s]
    out = nc.dram_tensor("out", [NG, PSL, G, T, FF, S], f32,
                         kind="ExternalOutput").ap()

    with tile.TileContext(nc) as tc:
        with (
            tc.tile_pool(name="const", bufs=1) as const,
            tc.tile_pool(name="w", bufs=W_BUFS) as wpool,
            tc.tile_pool(name="o", bufs=3) as opool,
            tc.tile_pool(name="psum", bufs=8, space="PSUM") as psum,
        ):
            # combined [adjB | adjA] mask: raw u8 over the fast HWDGE ring,
            # then one DVE cast-copy (also the same-engine funnel for the
            # mask TTs).  SWDGE cast-DMAs would cost ~17us of ramp-in.
            adjU = const.tile([128, K * S], mybir.dt.uint8)
            nc.sync.dma_start(adjU[:, :S], adj_xu[:])
            nc.sync.dma_start(
                adjU[:, S:].rearrange("p (h s) -> p h s", h=HS),
                adj_xx.rearrange("h p s -> p h s"))
            adjC = const.tile([128, K * S], IO_DT)
            nc.vector.tensor_copy(adjC[:], adjU[:])
            adjC_b = adjC[:, None, :].to_broadcast([128, CH, K * S])

            for g in range(NG):
                halves = []
                for hf in range(2):
                    hb = wpool.tile([128, WF + AF], IO_DT, tag="hb",
                                    name=f"hb_{g}_{hf}")
                    nc.sync.dma_start(hb[:], blob[g, hf])
                    wv = hb[:, :WF].rearrange("p (c k s) -> p c k s",
                                              c=CH, k=K)
                    av = hb[:, WF:].rearrange("p (c k g) -> p c k g",
                                              c=CH, k=K)
                    # mask B+A weights with ONE in-place multiply
                    nc.vector.tensor_tensor(
                        hb[:, :WF].rearrange("p (c ks) -> p c ks", c=CH),
                        hb[:, :WF].rearrange("p (c ks) -> p c ks", c=CH),
                        adjC_b, mybir.AluOpType.mult)
                    halves.append((wv, av))

                ps_tiles = [psum.tile([128, FF * S], f32, tag="ps",
                                      name=f"ps_{g}_{t}")
                            for t in range(T)]
                for c in range(CT):
                    hf, ci = divmod(c, CH)
                    wv, av = halves[hf]
                    t, r2 = divmod(c, CPT)
                    sl, cf = divmod(r2, FF)
                    pslice = ps_tiles[t][sl * 64:sl * 64 + G,
                                         cf * S:cf * S + S]
                    for k in range(K):
                        nc.tensor.matmul(
                            pslice,
                            lhsT=av[:, ci, k, :],
                            rhs=wv[:, ci, k, :],
                            start=(k == 0), stop=(k == K - 1))
                out_sb = opool.tile([128, T, FF, S], f32)
                for t in range(T):
                    for sl in range(PSL):
                        nc.scalar.copy(
                            out_sb[sl * 64:sl * 64 + G, t].rearrange(
                                "p f s -> p (f s)"),
                            ps_tiles[t][sl * 64:sl * 64 + G, :])
                for sl in range(PSL):
                    nc.scalar.dma_start(
                        out[g, sl], out_sb[sl * 64:sl * 64 + G])

    nc.compile()
    return nc


def kernel(x, u, WA, WB, adj_xx, adj_xu, context, _trace=False):
    B, S = x.shape
    _, A = u.shape
    C = WA.shape[0]
    assert C % N_CORES == 0
    CP = C // N_CORES
    HS = S // 128
    K = HS + 1
    NG = CP // CT
    CH = CT // 2

    # ---- host-side shard: group samples by context --------------------
    context = np.asarray(context)
    cnt = np.bincount(context, minlength=C)
    G = int(cnt.max())
    G = max(4, ((G + 3) // 4) * 4)
    order = np.argsort(context, kind="stable")
    starts = np.zeros(C + 1, np.int64)
    starts[1:] = np.cumsum(cnt)
    j = np.arange(G)
    valid = j[None, :] < cnt[:, None]                      # [C, G]
    pos = starts[:-1, None] + np.minimum(j[None, :],
                                         np.maximum(cnt[:, None] - 1, 0))
    gidx = order[pos]                                      # [C, G]

    Xp = np.asarray(x, np.float32)[gidx]                   # [C, G, S]
    Up = np.asarray(u, np.float32)[gidx]                   # [C, G, A]
    XpT = np.ascontiguousarray(Xp.transpose(0, 2, 1))      # [C, S, G]
    UpT = np.ascontiguousarray(Up.transpose(0, 2, 1))      # [C, A, G]

    WA = np.ascontiguousarray(WA, np.float32)
    WB = np.ascontiguousarray(WB, np.float32)
    adjxx_u8 = np.ascontiguousarray(adj_xx).view(np.uint8).reshape(HS, 128, S)
    adjxu_u8 = np.ascontiguousarray(adj_xu).view(np.uint8)

    WF = CH * K * S
    AF = CH * K * G
    in_maps = []
    for k in range(N_CORES):
        sl = slice(k * CP, (k + 1) * CP)
        # pack each half-group's weights+activations into one contiguous
        # blob: per partition line [CH, K, S] weights then [CH, K, G] acts,
        # slot 0 = B-term, slot 1+h = A-term K-chunk h
        W3 = np.empty((NG, 2, 128, CH, K, S), np.float32)
        W3[..., 0, :] = WB[sl].reshape(NG, 2, CH, 128, S) \
            .transpose(0, 1, 3, 2, 4)
        W3[..., 1:, :] = WA[sl].reshape(NG, 2, CH, HS, 128, S) \
            .transpose(0, 1, 4, 2, 3, 5)
        A3 = np.empty((NG, 2, 128, CH, K, G), np.float32)
        A3[..., 0, :] = UpT[sl].reshape(NG, 2, CH, 128, G) \
            .transpose(0, 1, 3, 2, 4)
        A3[..., 1:, :] = XpT[sl].reshape(NG, 2, CH, HS, 128, G) \
            .transpose(0, 1, 4, 2, 3, 5)
        blob = np.concatenate(
            [W3.reshape(NG, 2, 128, WF), A3.reshape(NG, 2, 128, AF)],
            axis=-1)
        in_maps.append({
            "blob": np.ascontiguousarray(blob).astype(ml_dtypes.bfloat16),
            "adj_xx": adjxx_u8,
            "adj_xu": adjxu_u8,
        })

    if _trace:
        _install_profile_shim()
    nc = _build_program(CP, S, A, G)
    res = run_bass_kernel_spmd(nc, in_maps, core_ids=list(range(N_CORES)),
                               trace=_trace)

    # device output blobs [NG, PSL, G, T, FF, S] -> [CP, G, S].
    # context c in a group lives at bank t=c//CPT, partition slot
    # sl=(c%CPT)//FF (64-aligned), free half cf=c%FF.
    outs = []
    for r in res.results:
        v = r["out"]
        # axes (g, sl, gg, t, cf, s) -> (g, t, sl, cf, gg, s)
        v = v.transpose(0, 3, 1, 4, 2, 5).reshape(CP, G, S)
        outs.append(v)
    Out_all = np.concatenate(outs, axis=0)                 # [C, G, S]
    out_full = np.zeros((B, S), np.float32)
    out_full[gidx[valid]] = Out_all[valid]

    if _trace:
        return out_full, res
    return out_full



# revision 8
# speedup vs baseline: 1.9361x; 1.1870x over previous
"""Trainium2 Bass kernel for ContextHyperLinearSSM.

Computes out[b,:] = x[b,:] @ (WA[context[b]] * adj_xx) + u[b,:] @ (WB[context[b]] * adj_xu)

Strategy: shard the CONTEXT axis across the 8 cores.  The host sorts
contexts by sample count (descending) and deals them round-robin, so
rank j on every core has a similar count and one SPMD program (with a
per-rank padded group size G_j baked in at build time) fits all cores.
Each core streams its 64 contexts' weight banks from HBM exactly once
in bf16, applies the adjacency masks on-device, and runs 3 accumulating
matmuls per context (f32 PSUM).  Each sample's row is computed by
exactly one core, so the host-side unshard is a pure scatter.

Device-side layout: contexts are processed in groups of CT=8; a group's
payload (weights + gathered/transposed x,u activations for all 8
contexts) is packed by the host into one contiguous HBM slab so a
single DMA per group moves 128 x ~13KB descriptors at full efficiency.
One in-place DVE multiply per half-group masks the weights against a
combined [adjB|adjA] bf16 tile.  All 8 contexts of a group accumulate
into 2 PSUM banks (2 x 64-aligned partition slots x 2 free halves);
ACT drains each bank slot with a f32->bf16 cast copy and the GpSimd
engine issues the output DMA.
"""

import sys

sys.path.insert(0, "/opt/trn_rl_repo")

import ml_dtypes
import numpy as np

import concourse.bass as bass
import concourse.mybir as mybir
import concourse.tile as tile
from concourse import bacc
from concourse.bass_utils import run_bass_kernel_spmd

N_CORES = 8
CT = 8   # contexts per group (one DMA + one PSUM generation)
CH = 4   # contexts per mask half
W_BUFS = 5

# blob / matmul operand dtype: bf16 halves HBM traffic and streams the PE
# at 4x the f32 rate; PSUM still accumulates in f32 (rel err ~3e-3 << 2e-2)
IO_DT = mybir.dt.bfloat16


def _install_profile_shim():
    """Register the NTFF profile hook that trn_boot skips when
    antenv.axon_hooks is missing from the image (profiling only)."""
    import types
    if "antenv.axon_hooks" in sys.modules:
        return
    try:
        from trn_agent_boot.trn_boot import _ntff_profile_via_ctypes
        hook = _ntff_profile_via_ctypes("/opt/axon/libaxon_pjrt.so")
    except Exception:
        hook = None
    mod = types.ModuleType("antenv.axon_hooks")
    mod.get_axon_ntff_profile_hook = lambda: hook
    mod.set_axon_ntff_profile_hook = lambda h: None
    sys.modules["antenv.axon_hooks"] = mod


def _layout(S, A, Gs):
    """Static blob/psum/output layout for per-rank group sizes Gs[64].

    Per group g the HBM slab holds, per partition line:
      [W_h0 (CH*K*S) | A_h0 (K*sum G) | W_h1 | A_h1]
    Returns per-group column offsets and per-(g,sl) output row offsets.
    """
    HS = S // 128
    K = HS + 1
    WF = CH * K * S
    CP = len(Gs)
    NG = CP // CT
    goff, aoffs, lws = [], {}, []
    col = 0
    for g in range(NG):
        goff.append(col)
        w = 0
        for h in range(2):
            w += WF
            for ci in range(CH):
                j = g * CT + h * CH + ci
                for kk in range(K):
                    aoffs[(j, kk)] = w
                    w += Gs[j]
        lws.append(w)
        col += w
    # output rows: per (g, sl) slab of R_gsl = Gs[g*CT + sl*2] rows
    rowoff = {}
    r = 0
    for g in range(NG):
        for sl in range(2):
            rowoff[(g, sl)] = r
            r += Gs[g * CT + sl * 2]
    return K, WF, NG, goff, aoffs, lws, rowoff, r


def _build_program(S, A, Gs):
    """Build the per-core Bass program. Gs = per-rank padded group sizes."""
    f32 = mybir.dt.float32
    nc = bacc.Bacc("TRN2", target_bir_lowering=False)

    assert S % 128 == 0 and A == 128
    K, WF, NG, goff, aoffs, lws, rowoff, totr = _layout(S, A, Gs)
    HS = K - 1
    LWmax = max(lws)
    TOT = sum(lws)
    FF = 2   # contexts along the free dim of a PSUM bank (512 f32 / S)
    T = 2    # PSUM banks per group

    blob = nc.dram_tensor("blob", [128, TOT], IO_DT,
                          kind="ExternalInput").ap()
    adjm = nc.dram_tensor("adjm", [128, K * S], IO_DT,
                          kind="ExternalInput").ap()
    out = nc.dram_tensor("out", [totr, T * FF * S], IO_DT,
                         kind="ExternalOutput").ap()

    with tile.TileContext(nc) as tc:
        with (
            tc.tile_pool(name="const", bufs=1) as const,
            tc.tile_pool(name="w", bufs=W_BUFS) as wpool,
            tc.tile_pool(name="o", bufs=3) as opool,
            tc.tile_pool(name="psum", bufs=8, space="PSUM") as psum,
        ):
            # combined [adjB | adjA] mask, host-prepared bf16 lines
            adjC = const.tile([128, K * S], IO_DT)
            nc.sync.dma_start(adjC[:], adjm[:])
            adjC_b = adjC[:, None, :].to_broadcast([128, CH, K * S])

            for g in range(NG):
                gt = wpool.tile([128, LWmax], IO_DT, tag="gb",
                                name=f"gb_{g}")
                nc.sync.dma_start(gt[:, :lws[g]],
                                  blob[:, goff[g]:goff[g] + lws[g]])
                ps_tiles = [psum.tile([128, FF * S], f32, tag="ps",
                                      name=f"ps_{g}_{t}")
                            for t in range(T)]
                for h in range(2):
                    woff = aoffs[(g * CT + h * CH, 0)] - WF
                    wv = gt[:, woff:woff + WF].rearrange(
                        "p (c ks) -> p c ks", c=CH)
                    # mask this half's B+A weights with ONE in-place multiply
                    nc.vector.tensor_tensor(wv, wv, adjC_b,
                                            mybir.AluOpType.mult)
                    for ci in range(CH):
                        c = h * CH + ci
                        j = g * CT + c
                        Gj = Gs[j]
                        t, r2 = divmod(c, 4)
                        sl, cf = divmod(r2, 2)
                        pslice = ps_tiles[t][sl * 64:sl * 64 + Gj,
                                             cf * S:cf * S + S]
                        for kk in range(K):
                            ao = aoffs[(j, kk)]
                            nc.tensor.matmul(
                                pslice,
                                lhsT=gt[:, ao:ao + Gj],
                                rhs=gt[:, woff + (ci * K + kk) * S:
                                       woff + (ci * K + kk) * S + S],
                                start=(kk == 0), stop=(kk == K - 1))
                out_sb = opool.tile([128, T, FF, S], IO_DT)
                for t in range(T):
                    for sl in range(2):
                        R = Gs[g * CT + t * 4 + sl * 2]
                        nc.scalar.copy(
                            out_sb[sl * 64:sl * 64 + R, t].rearrange(
                                "p f s -> p (f s)"),
                            ps_tiles[t][sl * 64:sl * 64 + R, :])
                for sl in range(2):
                    R = Gs[g * CT + sl * 2]
                    ro = rowoff[(g, sl)]
                    nc.scalar.dma_start(
                        out[ro:ro + R, :],
                        out_sb[sl * 64:sl * 64 + R].rearrange(
                            "p t f s -> p (t f s)"))

    nc.compile()
    return nc


def kernel(x, u, WA, WB, adj_xx, adj_xu, context, _trace=False):
    B, S = x.shape
    _, A = u.shape
    C = WA.shape[0]
    assert C % N_CORES == 0
    CP = C // N_CORES
    HS = S // 128
    K = HS + 1

    # ---- host-side shard: sort contexts by count, deal round-robin ----
    context = np.asarray(context)
    cnt = np.bincount(context, minlength=C)
    order_desc = np.argsort(-cnt, kind="stable")        # context ids
    # padded group size per per-core rank j: max count among the 8 cores
    # at that rank (= the first of the slice, counts sorted desc)
    Gs = [max(2, int(-2 * (-cnt[order_desc[j * N_CORES]] // 2)))
          for j in range(CP)]
    assert max(Gs) <= 64

    order = np.argsort(context, kind="stable")
    starts = np.zeros(C + 1, np.int64)
    starts[1:] = np.cumsum(cnt)

    Xf = np.asarray(x, np.float32)
    Uf = np.asarray(u, np.float32)
    WA = np.ascontiguousarray(WA, np.float32)
    WB = np.ascontiguousarray(WB, np.float32)

    K_, WF, NG, goff, aoffs, lws, rowoff, totr = _layout(S, A, Gs)

    # combined mask lines: [adj_xu[p,:] | adj_xx[p,:] (chunk0) | ...]
    adjm = np.empty((128, K * S), np.float32)
    adjm[:, :S] = np.asarray(adj_xu, np.float32)
    for h in range(HS):
        adjm[:, (1 + h) * S:(2 + h) * S] = \
            np.asarray(adj_xx[h * 128:(h + 1) * 128, :], np.float32)
    adjm = adjm.astype(ml_dtypes.bfloat16)

    in_maps = []
    ctx_of = {}   # (core, rank) -> context id
    for k in range(N_CORES):
        blob = np.zeros((128, sum(lws)), np.float32)
        for j in range(CP):
            ctx = int(order_desc[j * N_CORES + k])
            ctx_of[(k, j)] = ctx
            Gj = Gs[j]
            n = int(cnt[ctx])
            ids = order[starts[ctx]:starts[ctx] + n]
            if n == 0:
                ids = np.zeros(1, np.int64)
            ids = ids[np.minimum(np.arange(Gj), len(ids) - 1)]
            g, c = divmod(j, CT)
            h, ci = divmod(c, CH)
            woff = goff[g] + aoffs[(g * CT + h * CH, 0)] - WF
            # weights: line p, slot kk=0 -> WB rows, kk=1+hh -> WA chunk hh
            wcol = woff + ci * K * S
            blob[:, wcol:wcol + S] = WB[ctx]
            for hh in range(HS):
                blob[:, wcol + (1 + hh) * S:wcol + (2 + hh) * S] = \
                    WA[ctx][hh * 128:(hh + 1) * 128, :]
            # activations: [A|S]-transposed sample gathers
            ao = goff[g] + aoffs[(j, 0)]
            blob[:, ao:ao + Gj] = Uf[ids].T
            XT = Xf[ids].T                              # [S, Gj]
            for hh in range(HS):
                ao = goff[g] + aoffs[(j, 1 + hh)]
                blob[:, ao:ao + Gj] = XT[hh * 128:(hh + 1) * 128, :]
        in_maps.append({
            "blob": blob.astype(ml_dtypes.bfloat16),
            "adjm": adjm,
        })

    if _trace:
        _install_profile_shim()
    nc = _build_program(S, A, Gs)
    res = run_bass_kernel_spmd(nc, in_maps, core_ids=list(range(N_CORES)),
                               trace=_trace)

    # device output [totr, T*FF*S]: per (g, sl) a slab of R rows; context
    # c = t*4 + sl*2 + cf lives at free block [t, cf] of its slab.
    out_full = np.zeros((B, S), np.float32)
    for k, r in enumerate(res.results):
        v = np.asarray(r["out"]).astype(np.float32)
        v = v.reshape(totr, 2, 2, S)                    # [row, t, cf, S]
        for g in range(NG):
            for sl in range(2):
                ro = rowoff[(g, sl)]
                for t in range(2):
                    for cf in range(2):
                        j = g * CT + t * 4 + sl * 2 + cf
                        ctx = ctx_of[(k, j)]
                        n = int(cnt[ctx])
                        if n == 0:
                            continue
                        ids = order[starts[ctx]:starts[ctx] + n]
                        out_full[ids] = v[ro:ro + n, t, cf, :]

    if _trace:
        return out_full, res
    return out_full
